# revision 31
# baseline (speedup 1.0000x reference)
"""Trainium2 Bass kernel for the FFTBlock problem (B=2, C=32, H=2688, W=128).

Math (reference):
  spatial  = relu(conv7x1_s7(x) + b_spatial)                        [B,C,384,W]
  spectral = irfft(relu(w_spectral @ rfft_concat(x) + b_spectral))  per (b,c,w)
  out = spatial + spectral

Transformation: rfft/irfft along H are linear maps, so with
  F: rfft matrix (imag-DC and imag-Nyquist rows are zero and dropped -> 2688
     rows), A = w_spectral @ F : [384, 2688] (output channels 193/385 =
     imag-DC/imag-Nyquist are multiplied by zero irfft columns and dropped ->
     384), G: [384, 384] irfft matrix (same columns dropped):
  spectral_col = G @ relu(A @ x_col + b)

Device plan (8 NeuronCores, W sharded 8 x 16):
  Launch 1 "fold":  core i computes A^T[336*i:336*(i+1), :] =
                    F[:, hslice]^T @ w_spectral^T; host concatenates.
  Launch 2 "main":  per core: GEMM1 conv[384, 1024] = A @ x_cols (fp32r),
                    bias+relu (ACT), GEMM2 spec[384, 1024] = G @ relu (fp32r),
                    spatial conv as GEMM [32, 224] @ [224, 12288] (bf16 - the
                    spatial branch is small-magnitude; bf16 error is negligible
                    vs output absmax), reorg via DRAM bounce, on-chip add.
fp32r runs at full PE rate for free dim >= 256, with ~1e-4 relative error.
"""

import os

import numpy as np
import ml_dtypes

import contextlib

import concourse.bacc as bacc
import concourse.mybir as mybir
import concourse.tile as tile
from concourse.bass_utils import run_bass_kernel_spmd
from concourse.alu_op_type import AluOpType


def _maybe_loop(tc, n):
    return tc.For_i(0, n, 1) if n > 1 else contextlib.nullcontext()

N_CORES = 8
B, C, H, W = 2, 32, 2688, 128
FREQ_IN = H // 2 + 1            # 1345
KF = H                          # 2688 usable rfft rows (2 zero rows dropped)
OUT_H = 384
FREQ_OUT = OUT_H // 2 + 1       # 193
MO = 2 * FREQ_OUT - 2           # 384 usable conv channels (2 dead dropped)
WS = W // N_CORES               # 16 width columns per core
NCOL = B * C * WS               # 1024 spectral columns per core
NSP = B * OUT_H * WS            # 12288 spatial columns per core
KSP = C * 7                     # 224 spatial reduction
HSL = H // N_CORES              # 336 fold output rows per core

F32 = mybir.dt.float32
F32R = mybir.dt.float32r
BF16 = mybir.dt.bfloat16
F16 = mybir.dt.float16
RELU = mybir.ActivationFunctionType.Relu
COPY = mybir.ActivationFunctionType.Copy

_cache = {}
LAST_EXEC_NS = None
LAST_FOLD_NS = None


def _trace_flag():
    return bool(int(os.environ.get("KERNEL_TRACE", "0")))


def _dft_constants():
    """F [2688, 2688] (rfft, ortho, dead rows dropped) and G [384, 384]
    (irfft, ortho, dead cols dropped)."""
    if "F" in _cache:
        return _cache["F"], _cache["G"]
    Fc = np.fft.rfft(np.eye(H), axis=0, norm="ortho")       # [1345, 2688]
    F = np.concatenate([Fc.real, Fc.imag[1:FREQ_IN - 1]], axis=0)
    F = np.ascontiguousarray(F, dtype=np.float32)           # [2688, 2688]
    G_re = np.fft.irfft(np.eye(FREQ_OUT), n=OUT_H, axis=0, norm="ortho")
    G_im = np.fft.irfft(1j * np.eye(FREQ_OUT), n=OUT_H, axis=0, norm="ortho")
    G = np.concatenate([G_re, G_im[:, 1:FREQ_OUT - 1]], axis=1)
    G = np.ascontiguousarray(G, dtype=np.float32)           # [384, 384]
    _cache["F"] = F
    _cache["G"] = G
    return F, G


def _spec_keep_idx():
    """Kept rfft rows (of the 2690 concat) / output channels (of the 386)."""
    keep_f = list(range(FREQ_IN)) + [FREQ_IN + k for k in range(1, FREQ_IN - 1)]
    keep_o = list(range(FREQ_OUT)) + [FREQ_OUT + k for k in range(1, FREQ_OUT - 1)]
    return np.array(keep_f), np.array(keep_o)


def _build_fold(loop_n=1):
    """Per core: at_sl[336, 384] = f_sl[2688, 336]^T @ w_t[2688, 384]."""
    key = ("fold", loop_n)
    if key in _cache:
        return _cache[key]
    nc = bacc.Bacc("TRN2", target_bir_lowering=False, debug=False,
                   num_devices=N_CORES)
    f_sl = nc.dram_tensor("f_sl", [KF, HSL], F16, kind="ExternalInput").ap()
    w_t = nc.dram_tensor("w_t", [KF, MO], F16, kind="ExternalInput").ap()
    at_sl = nc.dram_tensor("at_sl", [HSL, MO], F32, kind="ExternalOutput").ap()

    KT = KF // 128               # 21
    MT = (HSL + 127) // 128      # 3 (128, 128, 80)

    with tile.TileContext(nc) as tc:
        with tc.tile_pool(name="w", bufs=1) as wp, \
             tc.tile_pool(name="f", bufs=1) as fp, \
             tc.tile_pool(name="o", bufs=2) as op, \
             tc.tile_pool(name="ps", bufs=1, space="PSUM") as pp:
            # batched loads: one DMA covers several 128-row k-tiles, laid
            # side by side in the free dim of one wide SBUF tile
            CH = 7                      # k-tiles per DMA
            wt, ft = [], []
            for g in range(KT // CH):
                wg = wp.tile([128, CH * MO], F16, tag=f"wg{g}", name=f"wg{g}")
                nc.sync.dma_start(
                    wg[:], w_t[128 * CH * g:128 * CH * (g + 1), :]
                    .rearrange("(k p) m -> p k m", p=128))
                fg = fp.tile([128, CH * HSL], F16, tag=f"fg{g}", name=f"fg{g}")
                nc.sync.dma_start(
                    fg[:], f_sl[128 * CH * g:128 * CH * (g + 1), :]
                    .rearrange("(k p) m -> p k m", p=128))
                for j in range(CH):
                    wt.append(wg[:, MO * j:MO * (j + 1)])
                    ft.append(fg[:, HSL * j:HSL * (j + 1)])
            for m in range(MT):
                mp = min(128, HSL - 128 * m)
                ps = pp.tile([mp, MO], F32, tag="ps", name="ps")
                for k in range(KT):
                    nc.tensor.matmul(ps[:], ft[k][:, 128 * m:128 * m + mp],
                                     wt[k], start=(k == 0), stop=(k == KT - 1))
                ot = op.tile([mp, MO], F32, tag="ot", name="ot")
                nc.scalar.activation(ot[:], ps[:], COPY)
                nc.sync.dma_start(at_sl[128 * m:128 * m + mp, :], ot[:])
    nc.compile()
    _cache[key] = nc
    return nc


def _build_main(loop_n=1):
    """Per core main kernel; out[384, 1024] in (t, (b, c, w)) layout."""
    key = ("main", loop_n)
    if key in _cache:
        return _cache[key]
    nc = bacc.Bacc("TRN2", target_bir_lowering=False, debug=False,
                   num_devices=N_CORES)
    at = nc.dram_tensor("at", [H, MO], F16, kind="ExternalInput").ap()
    xt = nc.dram_tensor("xt", [H, NCOL], F16, kind="ExternalInput").ap()
    gt = nc.dram_tensor("gt", [MO, OUT_H], F16, kind="ExternalInput").ap()
    bspec = nc.dram_tensor("bspec", [MO, 1], F32, kind="ExternalInput").ap()
    wsp = nc.dram_tensor("wsp", [KSP, C], F16, kind="ExternalInput").ap()
    bsp = nc.dram_tensor("bsp", [C, 1], F32, kind="ExternalInput").ap()
    xsp = nc.dram_tensor("xsp", [KSP, NSP], F16, kind="ExternalInput").ap()
    out = nc.dram_tensor("out", [OUT_H, NCOL], F32, kind="ExternalOutput").ap()

    KT1 = H // 128               # 21 k-tiles for GEMM1
    MT1 = MO // 128              # 3 m-tiles
    NT = NCOL // 512             # 2 n-tiles
    MT2 = OUT_H // 128           # 3 m-tiles for GEMM2
    NSPC = NSP // 512            # 24 spatial chunks
    TCH = 512 // WS              # 32 t values per spatial chunk

    with tile.TileContext(nc) as tc:
        with tc.tile_pool(name="const", bufs=1) as cst, \
             tc.tile_pool(name="xtp", bufs=1) as xtp, \
             tc.tile_pool(name="xspp", bufs=1) as xspp, \
             tc.tile_pool(name="relu", bufs=1) as rlp, \
             tc.tile_pool(name="spsb", bufs=2) as spsb, \
             tc.tile_pool(name="outp", bufs=2) as outp, \
             tc.tile_pool(name="ps_g1", bufs=1, space="PSUM") as psg1, \
             _maybe_loop(tc, loop_n):

            # ---- weights first (small, batched DMAs), then xt stream ----
            CH = 7                      # k-tiles per batched DMA
            at_t = []
            at_dmas = []
            for g in range(KT1 // CH):
                ag = cst.tile([128, CH * MO], F16, tag=f"atg{g}", name=f"atg{g}")
                at_dmas.append((ag, g))
                for j in range(CH):
                    at_t.append(ag[:, MO * j:MO * (j + 1)])
            gt_big = cst.tile([128, MT2 * OUT_H], F16, tag="gt_big", name="gt_big")
            nc.sync.dma_start(gt_big[:],
                              gt[:].rearrange("(k p) m -> p k m", p=128))
            gt_t = [gt_big[:, OUT_H * k:OUT_H * (k + 1)] for k in range(MT2)]
            bspec_big = cst.tile([128, MT1], F32, tag="bspec_big", name="bspec_big")
            nc.sync.dma_start(bspec_big[:],
                              bspec[:].rearrange("(m p) one -> p m one", p=128))
            bspec_t = [bspec_big[:, m:m + 1] for m in range(MT1)]
            wsp1 = cst.tile([128, C], F16, tag="wsp1", name="wsp1")
            nc.sync.dma_start(wsp1[:], wsp[0:128, :])
            wsp2 = cst.tile([KSP - 128, C], F16, tag="wsp2", name="wsp2")
            nc.sync.dma_start(wsp2[:], wsp[128:KSP, :])
            bsp_t = cst.tile([C, 1], F32, tag="bsp", name="bsp")
            nc.sync.dma_start(bsp_t[:], bsp[:])

            XCH = 3                     # xt k-tiles per DMA
            xt_t = []
            xt_tiles = []
            for g in range(KT1 // XCH):
                xg = xtp.tile([128, XCH * NCOL], F16, tag=f"xtg{g}", name=f"xtg{g}")
                xt_tiles.append(xg)
                for j in range(XCH):
                    xt_t.append(xg[:, NCOL * j:NCOL * (j + 1)])
            def emit_at_dma(g):
                ag = at_dmas[g][0]
                nc.sync.dma_start(
                    ag[:], at[128 * CH * g:128 * CH * (g + 1), :]
                    .rearrange("(k p) m -> p k m", p=128))

            def emit_xt_dma(g):
                nc.sync.dma_start(
                    xt_tiles[g][:], xt[128 * XCH * g:128 * XCH * (g + 1), :]
                    .rearrange("(k p) m -> p k m", p=128))

            for kind, g in [("at", 0), ("xt", 0), ("xt", 1), ("at", 1),
                            ("xt", 2), ("at", 2), ("xt", 3)]:
                (emit_at_dma if kind == "at" else emit_xt_dma)(g)

            # ---- GEMM1: conv[384, 1024] = A @ x; m-outer, k-inner, both
            # n-slices per weight load ----
            relu_t = []
            for m in range(MT1):
                rt = rlp.tile([128, NCOL], F16, tag=f"relu{m}", name=f"relu{m}")
                relu_t.append(rt)
            # spatial branch: scatter straight into `out` (spec layout);
            # the spectral result is accumulated on top via CCE accum DMA
            out_tcw = out.rearrange("t (b c w) -> t b c w", b=B, c=C)
            GRP = 6                       # 512-col chunks per xsp load group
            NGRP = NSPC // GRP            # 4 groups
            GW = GRP * 512                # 3072 cols per group

            xsp_tiles = {}

            def xsp_load(gi):
                x1 = xspp.tile([128, GW], F16, tag=f"xsp1_{gi}", name=f"xsp1_{gi}")
                nc.sync.dma_start(x1[:], xsp[0:128, GW * gi:GW * (gi + 1)])
                x2 = xspp.tile([KSP - 128, GW], F16, tag=f"xsp2_{gi}",
                               name=f"xsp2_{gi}")
                nc.sync.dma_start(x2[:], xsp[128:KSP, GW * gi:GW * (gi + 1)])
                xsp_tiles[gi] = (x1, x2)

            def spatial_group(gi):
                x1, x2 = xsp_tiles[gi]
                sp = spsb.tile([C, GW], F32, tag="sp", name="sp")
                for j in range(GRP):
                    jsl = slice(512 * j, 512 * (j + 1))
                    ps = psg1.tile([C, 512], F32,
                                   tag=f"g1m{j % MT1}n{j % NT}", name="ps_sp")
                    nc.tensor.matmul(ps[:], wsp1[:], x1[:, jsl], start=True, stop=False)
                    nc.tensor.matmul(ps[:], wsp2[:], x2[:, jsl], start=False, stop=True)
                    if j % 2 == 0:
                        nc.scalar.activation(sp[:, jsl], ps[:], RELU, bias=bsp_t[:])
                    else:
                        # relu(x + b) fused on DVE: (ps + bias) max 0
                        nc.vector.tensor_scalar(sp[:, jsl], ps[:], bsp_t[:], 0.0,
                                                AluOpType.add, AluOpType.max)
                # group gi covers b = gi // 2, t-range of 192
                b_i = gi // (NGRP // B)
                t0 = (GRP * TCH) * (gi % (NGRP // B))
                dst = out_tcw[t0:t0 + GRP * TCH, b_i, :, :].transpose([1, 0, 2])
                nc.sync.dma_start(dst, sp[:].rearrange("c (t w) -> c t w", w=WS))

            # queue remaining xt loads, then all xsp loads
            for g in (4, 5, 6):
                emit_xt_dma(g)
            for gi in range(NGRP):
                xsp_load(gi)

            # GEMM1 k-outer over all (m, n): each xt k-tile is fully
            # consumed on arrival (6 matmuls), PE stays dense and warm
            ps_mn = {}
            for m in range(MT1):
                for n in range(NT):
                    ps_mn[(m, n)] = psg1.tile([128, 512], F32,
                                              tag=f"g1m{m}n{n}", name=f"g1m{m}n{n}")
            for k in range(KT1):
                for m in range(MT1):
                    msl = slice(128 * m, 128 * (m + 1))
                    for n in range(NT):
                        nc.tensor.matmul(ps_mn[(m, n)][:], at_t[k][:, msl],
                                         xt_t[k][:, 512 * n:512 * (n + 1)],
                                         start=(k == 0), stop=(k == KT1 - 1))
            for m in range(MT1):
                for n in range(NT):
                    nc.scalar.activation(relu_t[m][:, 512 * n:512 * (n + 1)],
                                         ps_mn[(m, n)][:], RELU, bias=bspec_t[m][:])
            g2_pairs = [(m2, n) for m2 in range(MT2) for n in range(NT)]

            def gemm2_pair(m2, n):
                m2sl = slice(128 * m2, 128 * (m2 + 1))
                t0 = 128 * m2
                nsl = slice(512 * n, 512 * (n + 1))
                ps2 = psg1.tile([128, 512], F32, tag=f"g1m{m2}n{n}", name="g2")
                for k in range(MT2):
                    nc.tensor.matmul(ps2[:], gt_t[k][:, m2sl],
                                     relu_t[k][:, nsl],
                                     start=(k == 0), stop=(k == MT2 - 1))
                so = outp.tile([128, 512], F32, tag="so", name="so")
                nc.vector.tensor_copy(so[:], ps2[:])
                nc.gpsimd.dma_start(out[t0:t0 + 128, nsl], so[:],
                                    accum_op=AluOpType.add)

            # all spatial scatters must land before any accumulate touches
            # the same region of `out`, so GEMM2 accumulation comes last
            for gi in range(NGRP):
                spatial_group(gi)
            for m2, n in g2_pairs:
                gemm2_pair(m2, n)


    nc.compile()
    _cache[key] = nc
    return nc


def kernel(x, w_spatial, b_spatial, w_spectral, b_spectral):
    x = np.ascontiguousarray(x, dtype=np.float32)
    w_spatial = np.asarray(w_spatial, dtype=np.float32)
    b_spatial = np.asarray(b_spatial, dtype=np.float32)
    w_spectral = np.asarray(w_spectral, dtype=np.float32)
    b_spectral = np.asarray(b_spectral, dtype=np.float32)

    F, G = _dft_constants()
    keep_f, keep_o = _spec_keep_idx()
    core_ids = list(range(N_CORES))
    tr = _trace_flag()

    # ---- launch 1: fold A^T = F^T @ W^T, sharded over H ----
    nc1 = _build_fold()
    w_t = np.ascontiguousarray(w_spectral[keep_o][:, keep_f].T).astype(np.float16)
    in1 = [{"f_sl": np.ascontiguousarray(F[:, HSL * i:HSL * (i + 1)]).astype(np.float16),
            "w_t": w_t} for i in core_ids]
    kw1 = {}
    if tr:
        d = os.environ.get("KERNEL_TRACE_DIR", "/tmp/ktrace") + "/fold"
        os.makedirs(d, exist_ok=True)
        kw1 = dict(trace=True, tmpdir=d)
    res1 = run_bass_kernel_spmd(nc1, in1, core_ids, **kw1)
    global LAST_FOLD_NS
    LAST_FOLD_NS = res1.exec_time_ns
    at_full = np.concatenate([res1.results[i]["at_sl"] for i in core_ids], axis=0)

    # ---- launch 2: main ----
    nc2 = _build_main()
    gt = np.ascontiguousarray(G.T).astype(np.float16)             # [384, 384]
    bspec = np.ascontiguousarray(b_spectral[keep_o].reshape(MO, 1))
    wsp = np.ascontiguousarray(
        w_spatial[:, :, :, 0].transpose(1, 2, 0).reshape(KSP, C)
    ).astype(np.float16)
    bsp = np.ascontiguousarray(b_spatial.reshape(C, 1))
    at16 = at_full.astype(np.float16)
    in2 = []
    for i in core_ids:
        xs = x[:, :, :, WS * i:WS * (i + 1)]                      # [B, C, H, WS]
        xti = np.ascontiguousarray(
            xs.transpose(2, 0, 1, 3).reshape(H, NCOL)).astype(np.float16)
        xspi = np.ascontiguousarray(
            xs.reshape(B, C, OUT_H, 7, WS).transpose(1, 3, 0, 2, 4)
            .reshape(KSP, NSP)).astype(np.float16)
        in2.append({"at": at16, "xt": xti, "gt": gt, "bspec": bspec,
                    "wsp": wsp, "bsp": bsp, "xsp": xspi})
    kw2 = {}
    if tr:
        d = os.environ.get("KERNEL_TRACE_DIR", "/tmp/ktrace") + "/main"
        os.makedirs(d, exist_ok=True)
        kw2 = dict(trace=True, tmpdir=d)
    res2 = run_bass_kernel_spmd(nc2, in2, core_ids, **kw2)
    global LAST_EXEC_NS
    LAST_EXEC_NS = res2.exec_time_ns

    # ---- unshard: per-core out [384, (b, c, ws)] -> [B, C, 384, W] ----
    outs = np.stack([res2.results[i]["out"].reshape(OUT_H, B, C, WS)
                     for i in core_ids], axis=3)                  # [384,B,C,8,WS]
    return np.ascontiguousarray(
        outs.reshape(OUT_H, B, C, W).transpose(1, 2, 0, 3)).astype(np.float32)


# revision 33
# speedup vs baseline: 1.0965x; 1.0965x over previous
"""Trainium2 Bass kernel for the FFTBlock problem (B=2, C=32, H=2688, W=128).

Math (reference):
  spatial  = relu(conv7x1_s7(x) + b_spatial)                        [B,C,384,W]
  spectral = irfft(relu(w_spectral @ rfft_concat(x) + b_spectral))  per (b,c,w)
  out = spatial + spectral

Transformation: rfft/irfft along H are linear maps, so with
  F: rfft matrix (imag-DC and imag-Nyquist rows are zero and dropped -> 2688
     rows), A = w_spectral @ F : [384, 2688] (output channels 193/385 =
     imag-DC/imag-Nyquist are multiplied by zero irfft columns and dropped ->
     384), G: [384, 384] irfft matrix (same columns dropped):
  spectral_col = G @ relu(A @ x_col + b)

Device plan (8 NeuronCores, W sharded 8 x 16):
  Launch 1 "fold":  core i computes A^T[336*i:336*(i+1), :] =
                    F[:, hslice]^T @ w_spectral^T; host concatenates.
  Launch 2 "main":  per core: GEMM1 conv[384, 1024] = A @ x_cols (fp32r),
                    bias+relu (ACT), GEMM2 spec[384, 1024] = G @ relu (fp32r),
                    spatial conv as GEMM [32, 224] @ [224, 12288] (bf16 - the
                    spatial branch is small-magnitude; bf16 error is negligible
                    vs output absmax), reorg via DRAM bounce, on-chip add.
fp32r runs at full PE rate for free dim >= 256, with ~1e-4 relative error.
"""

import os

import numpy as np
import ml_dtypes

import contextlib

import concourse.bacc as bacc
import concourse.mybir as mybir
import concourse.tile as tile
from concourse.bass_utils import run_bass_kernel_spmd
from concourse.alu_op_type import AluOpType


def _maybe_loop(tc, n):
    return tc.For_i(0, n, 1) if n > 1 else contextlib.nullcontext()

N_CORES = 8
B, C, H, W = 2, 32, 2688, 128
FREQ_IN = H // 2 + 1            # 1345
KF = H                          # 2688 usable rfft rows (2 zero rows dropped)
OUT_H = 384
FREQ_OUT = OUT_H // 2 + 1       # 193
MO = 2 * FREQ_OUT - 2           # 384 usable conv channels (2 dead dropped)
WS = W // N_CORES               # 16 width columns per core
NCOL = B * C * WS               # 1024 spectral columns per core
NSP = B * OUT_H * WS            # 12288 spatial columns per core
KSP = C * 7                     # 224 spatial reduction
HSL = H // N_CORES              # 336 fold output rows per core

F32 = mybir.dt.float32
F32R = mybir.dt.float32r
BF16 = mybir.dt.bfloat16
F16 = mybir.dt.float16
RELU = mybir.ActivationFunctionType.Relu
COPY = mybir.ActivationFunctionType.Copy

_cache = {}
LAST_EXEC_NS = None
LAST_FOLD_NS = None


def _trace_flag():
    return bool(int(os.environ.get("KERNEL_TRACE", "0")))


def _dft_constants():
    """F [2688, 2688] (rfft, ortho, dead rows dropped) and G [384, 384]
    (irfft, ortho, dead cols dropped)."""
    if "F" in _cache:
        return _cache["F"], _cache["G"]
    Fc = np.fft.rfft(np.eye(H), axis=0, norm="ortho")       # [1345, 2688]
    F = np.concatenate([Fc.real, Fc.imag[1:FREQ_IN - 1]], axis=0)
    F = np.ascontiguousarray(F, dtype=np.float32)           # [2688, 2688]
    G_re = np.fft.irfft(np.eye(FREQ_OUT), n=OUT_H, axis=0, norm="ortho")
    G_im = np.fft.irfft(1j * np.eye(FREQ_OUT), n=OUT_H, axis=0, norm="ortho")
    G = np.concatenate([G_re, G_im[:, 1:FREQ_OUT - 1]], axis=1)
    G = np.ascontiguousarray(G, dtype=np.float32)           # [384, 384]
    _cache["F"] = F
    _cache["G"] = G
    return F, G


def _spec_keep_idx():
    """Kept rfft rows (of the 2690 concat) / output channels (of the 386)."""
    keep_f = list(range(FREQ_IN)) + [FREQ_IN + k for k in range(1, FREQ_IN - 1)]
    keep_o = list(range(FREQ_OUT)) + [FREQ_OUT + k for k in range(1, FREQ_OUT - 1)]
    return np.array(keep_f), np.array(keep_o)


def _build_fold(loop_n=1):
    """Per core: at_sl[336, 384] = f_sl[2688, 336]^T @ w_t[2688, 384]."""
    key = ("fold", loop_n)
    if key in _cache:
        return _cache[key]
    nc = bacc.Bacc("TRN2", target_bir_lowering=False, debug=False,
                   num_devices=N_CORES)
    f_sl = nc.dram_tensor("f_sl", [KF, HSL], F16, kind="ExternalInput").ap()
    w_t = nc.dram_tensor("w_t", [KF, MO], F16, kind="ExternalInput").ap()
    at_sl = nc.dram_tensor("at_sl", [HSL, MO], F32, kind="ExternalOutput").ap()

    KT = KF // 128               # 21
    MT = (HSL + 127) // 128      # 3 (128, 128, 80)

    with tile.TileContext(nc) as tc:
        with tc.tile_pool(name="w", bufs=1) as wp, \
             tc.tile_pool(name="f", bufs=1) as fp, \
             tc.tile_pool(name="o", bufs=2) as op, \
             tc.tile_pool(name="ps", bufs=1, space="PSUM") as pp:
            # batched loads: one DMA covers several 128-row k-tiles, laid
            # side by side in the free dim of one wide SBUF tile
            CH = 7                      # k-tiles per DMA
            wt, ft = [], []
            for g in range(KT // CH):
                wg = wp.tile([128, CH * MO], F16, tag=f"wg{g}", name=f"wg{g}")
                nc.sync.dma_start(
                    wg[:], w_t[128 * CH * g:128 * CH * (g + 1), :]
                    .rearrange("(k p) m -> p k m", p=128))
                fg = fp.tile([128, CH * HSL], F16, tag=f"fg{g}", name=f"fg{g}")
                nc.sync.dma_start(
                    fg[:], f_sl[128 * CH * g:128 * CH * (g + 1), :]
                    .rearrange("(k p) m -> p k m", p=128))
                for j in range(CH):
                    wt.append(wg[:, MO * j:MO * (j + 1)])
                    ft.append(fg[:, HSL * j:HSL * (j + 1)])
            for m in range(MT):
                mp = min(128, HSL - 128 * m)
                ps = pp.tile([mp, MO], F32, tag="ps", name="ps")
                for k in range(KT):
                    nc.tensor.matmul(ps[:], ft[k][:, 128 * m:128 * m + mp],
                                     wt[k], start=(k == 0), stop=(k == KT - 1))
                ot = op.tile([mp, MO], F32, tag="ot", name="ot")
                nc.scalar.activation(ot[:], ps[:], COPY)
                nc.sync.dma_start(at_sl[128 * m:128 * m + mp, :], ot[:])
    nc.compile()
    _cache[key] = nc
    return nc


def _build_main(loop_n=1):
    """Per core main kernel; out[384, 1024] in (t, (b, c, w)) layout."""
    key = ("main", loop_n)
    if key in _cache:
        return _cache[key]
    nc = bacc.Bacc("TRN2", target_bir_lowering=False, debug=False,
                   num_devices=N_CORES)
    at = nc.dram_tensor("at", [H, MO], F16, kind="ExternalInput").ap()
    xt = nc.dram_tensor("xt", [H, NCOL], F16, kind="ExternalInput").ap()
    gt = nc.dram_tensor("gt", [MO, OUT_H], F16, kind="ExternalInput").ap()
    bspec = nc.dram_tensor("bspec", [MO, 1], F32, kind="ExternalInput").ap()
    wsp = nc.dram_tensor("wsp", [KSP, C], F16, kind="ExternalInput").ap()
    bsp = nc.dram_tensor("bsp", [C, 1], F32, kind="ExternalInput").ap()
    xsp = nc.dram_tensor("xsp", [KSP, NSP], F16, kind="ExternalInput").ap()
    out = nc.dram_tensor("out", [OUT_H, NCOL], F32, kind="ExternalOutput").ap()

    KT1 = H // 128               # 21 k-tiles for GEMM1
    MT1 = MO // 128              # 3 m-tiles
    NT = NCOL // 512             # 2 n-tiles
    MT2 = OUT_H // 128           # 3 m-tiles for GEMM2
    NSPC = NSP // 512            # 24 spatial chunks
    TCH = 512 // WS              # 32 t values per spatial chunk

    with tile.TileContext(nc) as tc:
        with tc.tile_pool(name="const", bufs=1) as cst, \
             tc.tile_pool(name="xtp", bufs=1) as xtp, \
             tc.tile_pool(name="xspp", bufs=1) as xspp, \
             tc.tile_pool(name="relu", bufs=1) as rlp, \
             tc.tile_pool(name="spsb", bufs=2) as spsb, \
             tc.tile_pool(name="outp", bufs=2) as outp, \
             tc.tile_pool(name="ps_g1", bufs=1, space="PSUM") as psg1, \
             _maybe_loop(tc, loop_n):

            # ---- weights first (small, batched DMAs), then xt stream ----
            CH = 7                      # k-tiles per batched DMA
            at_t = []
            at_dmas = []
            for g in range(KT1 // CH):
                ag = cst.tile([128, CH * MO], F16, tag=f"atg{g}", name=f"atg{g}")
                at_dmas.append((ag, g))
                for j in range(CH):
                    at_t.append(ag[:, MO * j:MO * (j + 1)])
            gt_big = cst.tile([128, MT2 * OUT_H], F16, tag="gt_big", name="gt_big")
            nc.sync.dma_start(gt_big[:],
                              gt[:].rearrange("(k p) m -> p k m", p=128))
            gt_t = [gt_big[:, OUT_H * k:OUT_H * (k + 1)] for k in range(MT2)]
            bspec_big = cst.tile([128, MT1], F32, tag="bspec_big", name="bspec_big")
            nc.sync.dma_start(bspec_big[:],
                              bspec[:].rearrange("(m p) one -> p m one", p=128))
            bspec_t = [bspec_big[:, m:m + 1] for m in range(MT1)]
            wsp1 = cst.tile([128, C], F16, tag="wsp1", name="wsp1")
            nc.sync.dma_start(wsp1[:], wsp[0:128, :])
            wsp2 = cst.tile([KSP - 128, C], F16, tag="wsp2", name="wsp2")
            nc.sync.dma_start(wsp2[:], wsp[128:KSP, :])
            bsp_t = cst.tile([C, 1], F32, tag="bsp", name="bsp")
            nc.sync.dma_start(bsp_t[:], bsp[:])

            XCH = 3                     # xt k-tiles per DMA
            xt_t = []
            xt_tiles = []
            for g in range(KT1 // XCH):
                xg = xtp.tile([128, XCH * NCOL], F16, tag=f"xtg{g}", name=f"xtg{g}")
                xt_tiles.append(xg)
                for j in range(XCH):
                    xt_t.append(xg[:, NCOL * j:NCOL * (j + 1)])
            def emit_at_dma(g):
                ag = at_dmas[g][0]
                nc.sync.dma_start(
                    ag[:], at[128 * CH * g:128 * CH * (g + 1), :]
                    .rearrange("(k p) m -> p k m", p=128))

            def emit_xt_dma(g):
                nc.sync.dma_start(
                    xt_tiles[g][:], xt[128 * XCH * g:128 * XCH * (g + 1), :]
                    .rearrange("(k p) m -> p k m", p=128))

            for kind, g in [("at", 0), ("xt", 0), ("xt", 1), ("at", 1),
                            ("xt", 2), ("at", 2), ("xt", 3)]:
                (emit_at_dma if kind == "at" else emit_xt_dma)(g)

            # ---- GEMM1: conv[384, 1024] = A @ x; m-outer, k-inner, both
            # n-slices per weight load ----
            relu_t = []
            for m in range(MT1):
                rt = rlp.tile([128, NCOL], F16, tag=f"relu{m}", name=f"relu{m}")
                relu_t.append(rt)
            # spatial branch: scatter straight into `out` (spec layout);
            # the spectral result is accumulated on top via CCE accum DMA
            out_tcw = out.rearrange("t (b c w) -> t b c w", b=B, c=C)
            GRP = 6                       # 512-col chunks per xsp load group
            NGRP = NSPC // GRP            # 4 groups
            GW = GRP * 512                # 3072 cols per group

            xsp_tiles = {}

            def xsp_load(gi):
                x1 = xspp.tile([128, GW], F16, tag=f"xsp1_{gi}", name=f"xsp1_{gi}")
                nc.sync.dma_start(x1[:], xsp[0:128, GW * gi:GW * (gi + 1)])
                x2 = xspp.tile([KSP - 128, GW], F16, tag=f"xsp2_{gi}",
                               name=f"xsp2_{gi}")
                nc.sync.dma_start(x2[:], xsp[128:KSP, GW * gi:GW * (gi + 1)])
                xsp_tiles[gi] = (x1, x2)

            def spatial_group(gi):
                x1, x2 = xsp_tiles[gi]
                sp = spsb.tile([C, GW], F32, tag="sp", name="sp")
                for j in range(GRP):
                    jsl = slice(512 * j, 512 * (j + 1))
                    ps = psg1.tile([C, 512], F32,
                                   tag=f"g1m{j % MT1}n{j % NT}", name="ps_sp")
                    nc.tensor.matmul(ps[:], wsp1[:], x1[:, jsl], start=True, stop=False)
                    nc.tensor.matmul(ps[:], wsp2[:], x2[:, jsl], start=False, stop=True)
                    if j % 2 == 0:
                        nc.scalar.activation(sp[:, jsl], ps[:], RELU, bias=bsp_t[:])
                    else:
                        # relu(x + b) fused on DVE: (ps + bias) max 0
                        nc.vector.tensor_scalar(sp[:, jsl], ps[:], bsp_t[:], 0.0,
                                                AluOpType.add, AluOpType.max)
                # group gi covers b = gi // 2, t-range of 192
                b_i = gi // (NGRP // B)
                t0 = (GRP * TCH) * (gi % (NGRP // B))
                dst = out_tcw[t0:t0 + GRP * TCH, b_i, :, :].transpose([1, 0, 2])
                nc.sync.dma_start(dst, sp[:].rearrange("c (t w) -> c t w", w=WS))

            # queue remaining xt loads, then all xsp loads
            for g in (4, 5, 6):
                emit_xt_dma(g)
            for gi in range(NGRP):
                xsp_load(gi)

            # GEMM1 k-outer over all (m, n): each xt k-tile is fully
            # consumed on arrival (6 matmuls), PE stays dense and warm
            ps_mn = {}
            for m in range(MT1):
                for n in range(NT):
                    ps_mn[(m, n)] = psg1.tile([128, 512], F32,
                                              tag=f"g1m{m}n{n}", name=f"g1m{m}n{n}")
            for k in range(KT1):
                for m in range(MT1):
                    msl = slice(128 * m, 128 * (m + 1))
                    for n in range(NT):
                        nc.tensor.matmul(ps_mn[(m, n)][:], at_t[k][:, msl],
                                         xt_t[k][:, 512 * n:512 * (n + 1)],
                                         start=(k == 0), stop=(k == KT1 - 1))
            for m in range(MT1):
                for n in range(NT):
                    nc.scalar.activation(relu_t[m][:, 512 * n:512 * (n + 1)],
                                         ps_mn[(m, n)][:], RELU, bias=bspec_t[m][:])
            g2_pairs = [(m2, n) for m2 in range(MT2) for n in range(NT)]

            so_t = {}

            def gemm2_pair(m2, n):
                m2sl = slice(128 * m2, 128 * (m2 + 1))
                t0 = 128 * m2
                nsl = slice(512 * n, 512 * (n + 1))
                ps2 = psg1.tile([128, 512], F32, tag=f"g1m{m2}n{n}", name="g2")
                for k in range(MT2):
                    nc.tensor.matmul(ps2[:], gt_t[k][:, m2sl],
                                     relu_t[k][:, nsl],
                                     start=(k == 0), stop=(k == MT2 - 1))
                if m2 not in so_t:
                    so_t[m2] = outp.tile([128, NCOL], F32, tag=f"so{m2}",
                                         name=f"so{m2}")
                nc.vector.tensor_copy(so_t[m2][:, nsl], ps2[:])
                if n == NT - 1:
                    nc.gpsimd.dma_start(out[t0:t0 + 128, :], so_t[m2][:],
                                        accum_op=AluOpType.add)

            # an accumulate may only run after the spatial scatters covering
            # the same region of `out`. n indexes the batch half (cols =
            # b*512 + c*16 + w) and groups 0,1 / 2,3 cover b=0 / b=1, so
            # each half's accumulates overlap the other half's spatial work.
            spatial_group(0)
            spatial_group(1)
            for m2 in range(MT2):
                gemm2_pair(m2, 0)
            spatial_group(2)
            spatial_group(3)
            for m2 in range(MT2):
                gemm2_pair(m2, 1)


    nc.compile()
    _cache[key] = nc
    return nc


def kernel(x, w_spatial, b_spatial, w_spectral, b_spectral):
    x = np.ascontiguousarray(x, dtype=np.float32)
    w_spatial = np.asarray(w_spatial, dtype=np.float32)
    b_spatial = np.asarray(b_spatial, dtype=np.float32)
    w_spectral = np.asarray(w_spectral, dtype=np.float32)
    b_spectral = np.asarray(b_spectral, dtype=np.float32)

    F, G = _dft_constants()
    keep_f, keep_o = _spec_keep_idx()
    core_ids = list(range(N_CORES))
    tr = _trace_flag()

    # ---- launch 1: fold A^T = F^T @ W^T, sharded over H ----
    nc1 = _build_fold()
    w_t = np.ascontiguousarray(w_spectral[keep_o][:, keep_f].T).astype(np.float16)
    in1 = [{"f_sl": np.ascontiguousarray(F[:, HSL * i:HSL * (i + 1)]).astype(np.float16),
            "w_t": w_t} for i in core_ids]
    kw1 = {}
    if tr:
        d = os.environ.get("KERNEL_TRACE_DIR", "/tmp/ktrace") + "/fold"
        os.makedirs(d, exist_ok=True)
        kw1 = dict(trace=True, tmpdir=d)
    res1 = run_bass_kernel_spmd(nc1, in1, core_ids, **kw1)
    global LAST_FOLD_NS
    LAST_FOLD_NS = res1.exec_time_ns
    at_full = np.concatenate([res1.results[i]["at_sl"] for i in core_ids], axis=0)

    # ---- launch 2: main ----
    nc2 = _build_main()
    gt = np.ascontiguousarray(G.T).astype(np.float16)             # [384, 384]
    bspec = np.ascontiguousarray(b_spectral[keep_o].reshape(MO, 1))
    wsp = np.ascontiguousarray(
        w_spatial[:, :, :, 0].transpose(1, 2, 0).reshape(KSP, C)
    ).astype(np.float16)
    bsp = np.ascontiguousarray(b_spatial.reshape(C, 1))
    at16 = at_full.astype(np.float16)
    in2 = []
    for i in core_ids:
        xs = x[:, :, :, WS * i:WS * (i + 1)]                      # [B, C, H, WS]
        xti = np.ascontiguousarray(
            xs.transpose(2, 0, 1, 3).reshape(H, NCOL)).astype(np.float16)
        xspi = np.ascontiguousarray(
            xs.reshape(B, C, OUT_H, 7, WS).transpose(1, 3, 0, 2, 4)
            .reshape(KSP, NSP)).astype(np.float16)
        in2.append({"at": at16, "xt": xti, "gt": gt, "bspec": bspec,
                    "wsp": wsp, "bsp": bsp, "xsp": xspi})
    kw2 = {}
    if tr:
        d = os.environ.get("KERNEL_TRACE_DIR", "/tmp/ktrace") + "/main"
        os.makedirs(d, exist_ok=True)
        kw2 = dict(trace=True, tmpdir=d)
    res2 = run_bass_kernel_spmd(nc2, in2, core_ids, **kw2)
    global LAST_EXEC_NS
    LAST_EXEC_NS = res2.exec_time_ns

    # ---- unshard: per-core out [384, (b, c, ws)] -> [B, C, 384, W] ----
    outs = np.stack([res2.results[i]["out"].reshape(OUT_H, B, C, WS)
                     for i in core_ids], axis=3)                  # [384,B,C,8,WS]
    return np.ascontiguousarray(
        outs.reshape(OUT_H, B, C, W).transpose(1, 2, 0, 3)).astype(np.float32)
